# revision 5
# baseline (speedup 1.0000x reference)
"""CapsuleLayer (dynamic routing) Trainium2 Bass kernel.

Full inputs:  x [128, 512, 256] f32, W [32, 512, 16, 256] f32
Full output:  [128, 32, 16] f32

Sharding: split the input-capsule dim N=512 across 8 cores (64 each).
Each core computes its slice of inputs_hat = einsum('bni,mndi->bmnd'),
keeps it SBUF-resident as [b=128 part, (n_loc, m, d) free], runs the
3 routing iterations locally (softmax over m is fully local), and the
per-core partial s = sum_n c*inputs_hat is AllReduced (256KB) once per
iteration.  W and x are each read from HBM exactly once in aggregate
(~42MB per core), which is the memory roofline for this problem.
"""

import sys

sys.path.insert(0, "/opt/trn_rl_repo")

import numpy as np

import concourse.bacc as bacc
import concourse.mybir as mybir
import concourse.tile as tile
from concourse.bass_utils import run_bass_kernel_spmd

N_CORES = 8
B, N, I = 128, 512, 256
M, D = 32, 16
MD = M * D
NL = N // N_CORES          # 64 local input capsules per core
EPS = 1e-7
F32 = mybir.dt.float32

NB = 8                     # n-block size for xt DMA
CH = 8                     # n-chunk size for routing DVE passes


def _squash(tc, pool, s_src, scale_pre, eps_t):
    """o = squash(s) over d, s_src: [128, MD] AP (SBUF or PSUM). Returns o tile."""
    nc = tc.nc
    ssb = pool.tile([128, MD], F32, tag="ssb")
    nc.scalar.mul(out=ssb, in_=s_src, mul=scale_pre)  # copy (+scale) to SBUF
    sq = pool.tile([128, MD], F32, tag="sq")
    nc.vector.tensor_mul(sq, ssb, ssb)
    s2 = pool.tile([128, M], F32, tag="s2")
    nc.vector.tensor_reduce(
        s2, sq.rearrange("p (m d) -> p m d", d=D),
        axis=mybir.AxisListType.X, op=mybir.AluOpType.add,
    )
    rt = pool.tile([128, M], F32, tag="rt")
    nc.scalar.activation(rt, s2, mybir.ActivationFunctionType.Sqrt, bias=eps_t[:, 0:1])
    one_p = pool.tile([128, M], F32, tag="one_p")
    nc.vector.tensor_scalar_add(one_p, s2, 1.0)
    den = pool.tile([128, M], F32, tag="den")
    nc.vector.tensor_mul(den, one_p, rt)
    rec = pool.tile([128, M], F32, tag="rec")
    nc.vector.reciprocal(rec, den)
    scl = pool.tile([128, M], F32, tag="scl")
    nc.vector.tensor_mul(scl, s2, rec)      # scale = s2/(1+s2)/sqrt(s2+eps)
    o = pool.tile([128, MD], F32, tag="o")
    nc.vector.tensor_mul(
        o.rearrange("p (m d) -> p m d", d=D),
        ssb.rearrange("p (m d) -> p m d", d=D),
        scl.unsqueeze(2).broadcast_to([128, M, D]),
    )
    return o


def _allreduce(tc, dram_pool, sb_pool, src, idx):
    """AllReduce [128, MD] f32 across the 8 cores. Returns SBUF tile."""
    nc = tc.nc
    bin_ = dram_pool.tile([128, MD], F32, tag=f"arin{idx}")
    bout = dram_pool.tile([128, MD], F32, tag=f"arout{idx}")
    nc.sync.dma_start(out=bin_[:], in_=src)
    nc.gpsimd.collective_compute(
        "AllReduce", mybir.AluOpType.add,
        replica_groups=[list(range(N_CORES))],
        ins=[bin_.opt()], outs=[bout.opt()],
    )
    dst = sb_pool.tile([128, MD], F32, tag="sglob")
    nc.sync.dma_start(out=dst[:], in_=bout[:])
    return dst


def _body(tc, out_ap, wt, xt):
    nc = tc.nc
    X = mybir.AxisListType.X
    ADD = mybir.AluOpType.add

    with tc.tile_pool(name="persist", bufs=1) as persist, \
         tc.tile_pool(name="psum_s1", bufs=1, space="PSUM") as psum_s1, \
         tc.tile_pool(name="dram", bufs=1, space="DRAM") as dram:
        ih = persist.tile([128, NL, MD], F32)      # inputs_hat, 128KB/partition
        s1_ps = psum_s1.tile([128, MD], F32)       # sum_n inputs_hat (PE-accumulated)

        # ---------------- einsum phase ----------------
        with tc.tile_pool(name="xt_pool", bufs=3) as xt_pool, \
             tc.tile_pool(name="wt_pool", bufs=4) as wt_pool, \
             tc.tile_pool(name="psum_mm", bufs=4, space="PSUM") as psum_mm:
            for nb in range(NL // NB):
                xt_t = xt_pool.tile([128, 2, NB, B], F32)
                nc.sync.dma_start(
                    out=xt_t[:],
                    in_=xt[:, nb * NB:(nb + 1) * NB, :].rearrange(
                        "(h p) n b -> p h n b", p=128),
                )
                for j in range(NB):
                    n = nb * NB + j
                    wt_t = wt_pool.tile([128, 2, MD], F32)
                    nc.sync.dma_start(
                        out=wt_t[:],
                        in_=wt[n].rearrange("(h p) m -> p h m", p=128),
                    )
                    ps = psum_mm.tile([128, MD], F32)
                    nc.tensor.matmul(ps, lhsT=xt_t[:, 0, j, :], rhs=wt_t[:, 0, :],
                                     start=True, stop=False)
                    nc.tensor.matmul(ps, lhsT=xt_t[:, 1, j, :], rhs=wt_t[:, 1, :],
                                     start=False, stop=True)
                    # running sum over n for iteration-1 s (free on PE)
                    nc.tensor.matmul(s1_ps, lhsT=xt_t[:, 0, j, :], rhs=wt_t[:, 0, :],
                                     start=(n == 0), stop=False, skip_group_check=True)
                    nc.tensor.matmul(s1_ps, lhsT=xt_t[:, 1, j, :], rhs=wt_t[:, 1, :],
                                     start=False, stop=(n == NL - 1),
                                     skip_group_check=True)
                    nc.scalar.copy(out=ih[:, n, :], in_=ps)

        # ---------------- routing phase ----------------
        with tc.tile_pool(name="rp", bufs=1) as rp, \
             tc.tile_pool(name="rsmall", bufs=2) as rsmall, \
             tc.tile_pool(name="tmp_pool", bufs=1) as tmp_pool:
            eps_t = rp.tile([128, 1], F32, tag="eps")
            nc.vector.memset(eps_t, EPS)
            # iteration 1: c is uniform 1/M -> s1 = sum_n ih / M
            s1_sb = rsmall.tile([128, MD], F32, tag="s1_sb")
            nc.scalar.mul(out=s1_sb, in_=s1_ps[:], mul=1.0 / M)
            s1g = _allreduce(tc, dram, rsmall, s1_sb[:], 0)
            o = _squash(tc, rsmall, s1g[:], 1.0, eps_t)

            b_log = rp.tile([128, NL, M], F32)     # routing logits
            for it in (2, 3):
                # b-update: b_log (+)= sum_d o * ih
                for k in range(NL // CH):
                    ksl = slice(k * CH, (k + 1) * CH)
                    tmp = tmp_pool.tile([128, CH, MD], F32, tag="tmp")
                    nc.vector.tensor_mul(
                        tmp, ih[:, ksl, :],
                        o.unsqueeze(1).broadcast_to([128, CH, MD]),
                    )
                    t4 = tmp.rearrange("p n (m d) -> p n m d", d=D)
                    if it == 2:
                        nc.vector.tensor_reduce(b_log[:, ksl, :], t4, axis=X, op=ADD)
                    else:
                        bup = rsmall.tile([128, CH, M], F32, tag="bup")
                        nc.vector.tensor_reduce(bup, t4, axis=X, op=ADD)
                        nc.vector.tensor_add(b_log[:, ksl, :], b_log[:, ksl, :], bup)
                # softmax over m (free-dim inner): c = exp(b) / sum_m exp(b)
                e_t = rp.tile([128, NL, M], F32, tag="e_t")
                nc.scalar.activation(e_t, b_log, mybir.ActivationFunctionType.Exp)
                zt = rsmall.tile([128, NL], F32, tag="zt")
                nc.vector.tensor_reduce(zt, e_t, axis=X, op=ADD)
                rz = rsmall.tile([128, NL], F32, tag="rz")
                nc.vector.reciprocal(rz, zt)
                c_t = rp.tile([128, NL, M], F32, tag="c_t")
                nc.vector.tensor_mul(
                    c_t, e_t, rz.unsqueeze(2).broadcast_to([128, NL, M]))
                # s-step: s = sum_n c * ih   (partial over local n)
                s_acc = rsmall.tile([128, MD], F32, tag="s_acc")
                for k in range(NL // CH):
                    ksl = slice(k * CH, (k + 1) * CH)
                    tmp = tmp_pool.tile([128, CH, MD], F32, tag="tmp")
                    nc.vector.tensor_mul(
                        tmp.rearrange("p n (m d) -> p n m d", d=D),
                        ih[:, ksl, :].rearrange("p n (m d) -> p n m d", d=D),
                        c_t[:, ksl, :].unsqueeze(3).broadcast_to([128, CH, M, D]),
                    )
                    tT = tmp.rearrange("p n md -> p md n")
                    if k == 0:
                        nc.vector.tensor_reduce(s_acc, tT, axis=X, op=ADD)
                    else:
                        sp = rsmall.tile([128, MD], F32, tag="sp")
                        nc.vector.tensor_reduce(sp, tT, axis=X, op=ADD)
                        nc.vector.tensor_add(s_acc, s_acc, sp)
                sg = _allreduce(tc, dram, rsmall, s_acc[:], it - 1)
                o = _squash(tc, rsmall, sg[:], 1.0, eps_t)

            nc.sync.dma_start(out=out_ap, in_=o[:])


_cache = {}


def _build():
    if "nc" in _cache:
        return _cache["nc"]
    nc = bacc.Bacc("TRN2", target_bir_lowering=False, debug=False,
                   enable_asserts=True, num_devices=N_CORES)
    wt = nc.dram_tensor("wt", [NL, I, MD], F32, kind="ExternalInput").ap()
    xt = nc.dram_tensor("xt", [I, NL, B], F32, kind="ExternalInput").ap()
    out = nc.dram_tensor("out", [B, MD], F32, kind="ExternalOutput").ap()
    with tile.TileContext(nc) as tc:
        _body(tc, out, wt, xt)
    nc.compile()
    _cache["nc"] = nc
    return nc


def make_in_maps(x, W):
    """Host-side shard prep: per-core transposed views of x and W."""
    # WT[n, i, (m,d)] so rhs tiles [i', md] are contiguous per (n, ihalf)
    WT = np.ascontiguousarray(W.transpose(1, 3, 0, 2)).reshape(N, I, MD)
    # XT[i, n, b] so lhsT tiles [i', b] stream per n-block
    XT = np.ascontiguousarray(x.transpose(2, 1, 0))
    in_maps = []
    for c in range(N_CORES):
        sl = slice(c * NL, (c + 1) * NL)
        in_maps.append({
            "wt": WT[sl],                                   # contiguous view
            "xt": np.ascontiguousarray(XT[:, sl, :]),
        })
    return in_maps


def kernel(x, W, _trace=False):
    x = np.asarray(x, dtype=np.float32)
    W = np.asarray(W, dtype=np.float32)
    nc = _build()
    in_maps = make_in_maps(x, W)
    res = run_bass_kernel_spmd(nc, in_maps, core_ids=list(range(N_CORES)),
                               trace=_trace)
    _cache["last_result"] = res
    return res.results[0]["out"].reshape(B, M, D)
